# revision 7
# baseline (speedup 1.0000x reference)
"""Bass/Trainium2 kernel v4 for nn_EnhancedCircuitLoss — DMA-bound stream.

Math: the O(B*L^2*D) dep term collapses to per-batch S = sum_l sq_l e_l and
Q = sum_l sq_l^2 |e_l|^2 via sum_b sq.dep = (|S_b|^2 - Q_b)/2.

v4 (31.4us/core, from 46.7us v3). The 8MB/core fp8 embedding stream is the
roofline (23.3us at 360 B/ns); everything else hides under it:
- emb stored host-side as fp8(e/4) so every byte has exponent < 8; batches
  arrive in 4-batch DMAs (tapered 2/1/quartered at the tail) — 13 input
  DMAs total instead of 35+, so HWDGE/SEQ overheads and the per-DMA
  completion sems stay off the stream.
- Q squares: one DVE tensor_scalar logical_shift_left(1) on a uint16 view
  per batch (422ns at the 4x DVE rate). Doubling fp8 bit patterns squares
  the values up to a 2^7 scale (folded into host combine) with <=9%
  elementwise error (mantissa interpolation + cross-byte carry), far
  inside this term's error budget; e/4 keeps 2E < 16 so no sign/NaN
  overflow. The e2' stream feeds DoubleRow wred matmuls (wt sq^2/16)
  accumulated across all batches in one PSUM bank (row 0 read once).
- S: DoubleRow matmuls with guard-spaced scattered sq weights (stride 68
  per batch, 16 per chunk-pair, double-buffered 784-col regions) so each
  8-batch group's S vectors land on DISTINCT PSUM partitions (0..7) of
  one accumulated bank; |S|^2 is a single [8,256] ACT Square+accum per
  group, overlapped with the stream (last group parallel with the DVE
  qwr drain).
- small terms in the flat [128, 2, 257] overlap layout (boundary dups);
  weight prep from an fp8 [128, 2, 256] (k, (b,c)) layout — no PE
  transposes, no big memsets (zeros live once in the 2x784 guard tile).
"""

import numpy as np
import ml_dtypes

import concourse.bacc as bacc
import concourse.bass as bass
import concourse.mybir as mybir
import concourse.tile as tile
from concourse.bass_utils import run_bass_kernel_spmd

F32 = mybir.dt.float32
BF16 = mybir.dt.bfloat16
FP8 = mybir.dt.float8e4
U16 = mybir.dt.uint16
FP8NP = ml_dtypes.float8_e4m3
ALU = mybir.AluOpType
AF = mybir.ActivationFunctionType

B, L, D = 256, 1024, 256
NCORES = 8
BS = B // NCORES          # 32 batches per core
NCH = L // 128            # 8 l-chunks of 128
NG = NCH // 2             # 4 double-chunks for DoubleRow
EPS = 1e-6
FLAT = BS * L
FP_ = FLAT // 128         # 256 per partition

# emb stored as e/4 (keeps fp8 exponent < 8 so bit-doubling can't spill
# into the sign bit); squares come back as (e/4)^2 * 2^7 = e^2 / 2.
ESCALE = 4.0
S2SCALE = ESCALE * ESCALE          # |S|^2 = S2SCALE * raw
QSCALE = 2.0                       # Q = 16(sq2) * ESCALE^2 / 2^7 * raw

# DMA batch grouping: big groups for BW, tapered tail for latency.
GROUPS = [4, 4, 4, 4, 4, 4, 4, 2, 1, "q"]
assert sum(g if g != "q" else 1 for g in GROUPS) == BS

_CACHE = {}


def _build_nc():
    nc = bacc.Bacc("TRN2", target_bir_lowering=False, debug=False)

    emb = nc.dram_tensor("emb", [BS, 128, NCH * D], FP8, kind="ExternalInput")
    spat_t = nc.dram_tensor("spat_t", [128, 2 * FP_], FP8, kind="ExternalInput")
    spat_f = nc.dram_tensor("spat_f", [128, 2 * (FP_ + 1)], F32,
                            kind="ExternalInput")
    smalls = nc.dram_tensor("smalls", [BS, 6], F32, kind="ExternalInput")
    partials = nc.dram_tensor("partials", [128, 16], F32, kind="ExternalOutput")

    with tile.TileContext(nc) as tc:
        with (
            tc.tile_pool(name="persist", bufs=1) as pp,
            tc.tile_pool(name="ebuf", bufs=4) as ep,
            tc.tile_pool(name="e2buf", bufs=4) as e2p,
            tc.tile_pool(name="psum", bufs=1, space=bass.MemorySpace.PSUM) as ps,
        ):
            # re-init the one const AP the stripped preamble memsets fed
            # (activation bias 0.0); ordered before all ACT Squares via the
            # DVE->ACT sqd_t sem chain
            nc.vector.memset(nc.const_aps.aps[(F32, 0.0)], 0.0)

            # ---------- input DMAs (order = DMA device order) ----------
            spatt_b = pp.tile([128, 2, FP_], FP8, tag="spatt_b")
            spat_b = pp.tile([128, 2, FP_ + 1], F32, tag="spat_b")
            smalls_b = pp.tile([BS, 6], F32, tag="smalls_b")

            ebufs = []
            off = 0
            group_of_batch = {}
            for gi, gsz in enumerate(GROUPS):
                if gsz == "q":
                    # final batch: 4 quarter-DMAs (2 chunks each) so the
                    # tail shl/wred pipeline with the transfer
                    t = ep.tile([128, 1, NCH, D], FP8, tag="ebufq")
                    for h in range(4):
                        nc.sync.dma_start(
                            t[:, 0, 2 * h:2 * h + 2, :],
                            emb.ap()[off].rearrange(
                                "k (c d) -> k c d", c=NCH)[:, 2 * h:2 * h + 2, :])
                    group_of_batch[off] = (t, 0)
                    off += 1
                    continue
                t = ep.tile([128, gsz, NCH, D], FP8, tag=f"ebuf{gsz}")
                nc.sync.dma_start(
                    t[:], emb.ap()[off:off + gsz].rearrange(
                        "b k (c d) -> k b c d", c=NCH))
                for j in range(gsz):
                    group_of_batch[off + j] = (t, j)
                off += gsz
                if gi == 0:
                    nc.sync.dma_start(
                        spatt_b[:],
                        spat_t.ap().rearrange("p (s j) -> p s j", s=2))
                elif gi == 1:
                    nc.sync.dma_start(
                        spat_b[:],
                        spat_f.ap().rearrange("p (s j) -> p s j", s=2))
                    nc.sync.dma_start(smalls_b[:], smalls.ap())

            # ---------- weight prep ----------
            # sq8w: double-buffered guard-spaced scatter regions (784 each).
            # DoubleRow out row p reads lhsT cols [254-2p, 255-2p]; batch b8
            # of the active group lands its S row at partition b8: value
            # cols V = 254 + 66*b8 + 16*g + j, window x = 68*b8 + 16*g.
            sq8w = pp.tile([128, 2, 784], FP8, tag="sq8w")
            nc.gpsimd.memset(sq8w[:].rearrange("p a b -> p (a b)"), 0.0)
            # wred weights: sliding window x=8b+2g over [254 zeros | sq2T]
            sq2pad = pp.tile([128, 510], FP8, tag="sq2pad")
            nc.gpsimd.memset(sq2pad[:, 0:254], 0.0)

            sqd_t = pp.tile([128, FP_], BF16, tag="sqd_t")
            nc.vector.tensor_sub(sqd_t[:], spatt_b[:, 0, :], spatt_b[:, 1, :])
            sq8T = pp.tile([128, FP_], FP8, tag="sq8T")
            nc.scalar.activation(sq8T[:], sqd_t[:], AF.Square)
            nc.scalar.activation(sq2pad[:, 254:510], sq8T[:], AF.Square,
                                 scale=0.25)

            def scatter(q):
                # values at cols 240 + 66*b8 + 16*g + j of region q%2,
                # from sq8T cols 64q + 8*b8 + 2*g + j
                for j in range(2):
                    src = sq8T[:, 64 * q + j:64 * q + j + 63:2].rearrange(
                        "p (b g) -> p b g", b=8)
                    dst = sq8w[:, q % 2, 254 + j:254 + j + 66 * 8:1].rearrange(
                        "p (b r) -> p b r", b=8)[:, :, 0:49:16]
                    nc.vector.tensor_copy(dst, src)

            scatter(0)
            scatter(1)

            # ---------- persistent tiles for small terms ----------
            spB = spat_b[:, 0, :]
            atB = spat_b[:, 1, :]
            sqd_f = pp.tile([128, FP_], F32, tag="sqd_f")
            d_at = pp.tile([128, FP_], F32, tag="d_at")
            spe = pp.tile([128, FP_ + 1], F32, tag="spe")
            ate = pp.tile([128, FP_ + 1], F32, tag="ate")
            rsp = pp.tile([128, FP_], F32, tag="rsp")
            rat = pp.tile([128, FP_], F32, tag="rat")
            t1 = pp.tile([128, FP_], F32, tag="t1")
            t2 = pp.tile([128, FP_], F32, tag="t2")
            rdiff = pp.tile([128, FP_], F32, tag="rdiff")
            s2scr = pp.tile([128, 256], BF16, tag="s2scr")
            prodB = pp.tile([128, FP_], F32, tag="prodB")
            sq_scr = pp.tile([128, FP_], BF16, tag="sq_scr")

            finals = pp.tile([128, 16], F32, tag="finals")
            nc.gpsimd.memset(finals[:], 0.0)

            # DVE ops to interleave into stream gaps (cheap-first order
            # matters only for dependencies)
            dve_ops = [
                lambda: nc.vector.tensor_sub(sqd_f[:], spB[:, 1:], atB[:, 1:]),
                lambda: nc.vector.tensor_scalar(spe[:], spB[:], EPS, None,
                                                ALU.add),
                lambda: nc.vector.tensor_scalar(ate[:], atB[:], EPS, None,
                                                ALU.add),
                lambda: nc.vector.reciprocal(rsp[:], spe[:, 0:FP_]),
                lambda: nc.vector.reciprocal(rat[:], ate[:, 0:FP_]),
                lambda: nc.vector.tensor_mul(t1[:], spe[:, 1:], rsp[:]),
                lambda: nc.vector.tensor_mul(t2[:], ate[:, 1:], rat[:]),
                lambda: nc.vector.tensor_sub(rdiff[:], t1[:], t2[:]),
                lambda: nc.vector.tensor_sub(d_at[:], atB[:, 1:], atB[:, 0:FP_]),
                lambda: nc.vector.tensor_mul(prodB[:], sqd_f[:], d_at[:]),
                lambda: nc.vector.tensor_reduce(
                    finals[:, 3:4], sqd_f[:], mybir.AxisListType.X,
                    ALU.add, apply_absolute_value=True),
                lambda: nc.vector.tensor_reduce(
                    finals[:, 4:5], prodB[:], mybir.AxisListType.X,
                    ALU.add, apply_absolute_value=True),
            ]
            act_ops = {
                10: lambda: nc.scalar.activation(
                    sq_scr[:], sqd_f[:], AF.Square, accum_out=finals[:, 1:2]),
                20: lambda: nc.scalar.activation(
                    sq_scr[:], rdiff[:], AF.Square, accum_out=finals[:, 2:3]),
            }

            # Pool-engine small chain (off critical path)
            fd = pp.tile([BS, 1], F32, tag="fd")
            lg = pp.tile([BS, 1], F32, tag="lg")
            invu = pp.tile([BS, 1], F32, tag="invu")
            tJ = pp.tile([BS, 1], F32, tag="tJ")
            nc.scalar.activation(lg[:], smalls_b[:, 1:2], AF.Ln)
            nc.vector.reciprocal(invu[:], smalls_b[:, 1:2])
            nc.gpsimd.tensor_sub(fd[:], smalls_b[:, 0:1], smalls_b[:, 2:3])
            nc.gpsimd.tensor_mul(finals[0:BS, 0:1], fd[:], fd[:])
            nc.gpsimd.tensor_mul(finals[0:BS, 5:6], finals[0:BS, 0:1], invu[:])
            nc.gpsimd.tensor_add(finals[0:BS, 5:6], finals[0:BS, 5:6], lg[:])
            nc.gpsimd.tensor_sub(tJ[:], smalls_b[:, 3:4], smalls_b[:, 4:5])
            nc.vector.tensor_reduce(finals[0:BS, 11:12], tJ[:],
                                    mybir.AxisListType.X, ALU.add,
                                    apply_absolute_value=True)

            # ---------- stream ----------
            # full-bank (2KB) PSUM tiles so concurrent accumulation groups
            # never share a bank
            sbanks = []
            for q in range(4):
                sbank_t = ps.tile([128, 512], F32, tag=f"sbank{q}")
                sbanks.append(sbank_t[:, 0:256])
            qwr_t = ps.tile([128, 512], F32, tag="qwr")
            qwr = qwr_t[:, 0:256]

            for b in range(BS):
                t, j = group_of_batch[b]
                q = b // 8
                b8 = b % 8

                def s_mm(g):
                    x = 784 * (q % 2) + 68 * b8 + 16 * g
                    nc.tensor.matmul(
                        sbanks[q][:],
                        sq8w[:].rearrange("p a b -> p (a b)")[:, x:x + 256],
                        t[:, j, 2 * g:2 * g + 2, :],
                        start=(b8 == 0 and g == 0),
                        stop=(b8 == 7 and g == NG - 1),
                        perf_mode=mybir.MatmulPerfMode.DoubleRowSwInterleave,
                        skip_group_check=True)

                def wred_mm(g, e2):
                    x = 8 * b + 2 * g
                    nc.tensor.matmul(
                        qwr[:], sq2pad[:, x:x + 256],
                        e2[:, 2 * g:2 * g + 2, :],
                        start=(b == 0 and g == 0),
                        stop=(b == BS - 1 and g == NG - 1),
                        perf_mode=mybir.MatmulPerfMode.DoubleRowSwInterleave,
                        skip_group_check=True)

                e2 = e2p.tile([128, NCH, D], FP8, tag="e2")
                if b == BS - 1:
                    # quartered tail batch: S-mm, shl, wred per chunk-pair
                    for h in range(NG):
                        s_mm(h)
                        sl = slice(2 * h, 2 * h + 2)
                        nc.vector.tensor_scalar(
                            e2[:, sl].rearrange("p c d -> p (c d)").bitcast(U16),
                            t[:, j, sl].rearrange("p c d -> p (c d)").bitcast(U16),
                            1, None, ALU.logical_shift_left)
                        wred_mm(h, e2)
                else:
                    for g in range(NG):
                        s_mm(g)
                    nc.vector.tensor_scalar(
                        e2[:].rearrange("p c d -> p (c d)").bitcast(U16),
                        t[:, j].rearrange("p c d -> p (c d)").bitcast(U16),
                        1, None, ALU.logical_shift_left)
                    for g in range(NG):
                        wred_mm(g, e2)

                if b8 == 7:
                    # |S|^2 for the finished group: rows 0..7
                    nc.scalar.activation(
                        s2scr[0:8, :], sbanks[q][0:8, :], AF.Square,
                        accum_out=finals[0:8, 6 + q:7 + q])
                    if q + 2 < 4:
                        scatter(q + 2)

                if b >= 4 and dve_ops:
                    dve_ops.pop(0)()
                if b in act_ops:
                    act_ops.pop(b)()

            for f in dve_ops:
                f()
            for f in act_ops.values():
                f()

            # Q drain: row 0 of qwr, on DVE (parallel with ACT q3 square)
            nc.vector.tensor_reduce(finals[0:1, 10:11], qwr[0:1, :],
                                    mybir.AxisListType.X, ALU.add)
            nc.sync.dma_start(partials.ap(), finals[:])

    # strip all 4 const-AP preamble memsets (they serialize on Pool ahead of
    # the entry barrier); the one const this kernel reads (activation bias
    # 0.0) is re-initialized in the body on DVE, ordered before every ACT
    # Square by the DVE->ACT sqd_t sem chain
    blk0 = nc.m.functions[0].blocks[0]
    _memsets = [i for i in blk0.instructions if isinstance(i, mybir.InstMemset)][:4]
    for _ins in _memsets:
        blk0.instructions.remove(_ins)
    # hoist the first emb DMA (no data deps) ahead of the entry barrier so
    # its transfer overlaps the barrier round-trip
    _first_dma = None
    _blk_of = None
    for _blk in nc.m.functions[0].blocks:
        for _i in _blk.instructions:
            if isinstance(_i, mybir.InstDMACopy):
                _first_dma = _i
                _blk_of = _blk
                break
        if _first_dma is not None:
            break
    _blk_of.instructions.remove(_first_dma)
    blk0.instructions.insert(1, _first_dma)
    # drop the SECOND exit barrier round (drains + barriers after the Pool
    # sem-clear ISA); round 1 already orders the sem-clear after all engines'
    # completion waits, and exit-barrier sems are untouched either way
    _exit_blk = nc.m.functions[0].blocks[-1]
    _exit = list(_exit_blk.instructions)
    _isa_idx = max(i for i, x in enumerate(_exit)
                   if isinstance(x, mybir.InstISA))
    for _ins in _exit[_isa_idx + 1:]:
        _exit_blk.instructions.remove(_ins)
    nc.compile()
    # the DMA-queue completion waits are generated BY compile(); drop the SP
    # exit waits queued after the long output-DMA wait (input queues that
    # completed mid-stream + engine tick sems already guarded by the exit
    # drains) so the barrier+sem-clear start as soon as the output lands
    _exit_blk2 = nc.m.functions[0].blocks[-1]
    _exit2 = list(_exit_blk2.instructions)
    _sp_waits = [i for i, x in enumerate(_exit2)
                 if isinstance(x, mybir.InstEventSemaphore)
                 and x.engine == mybir.EngineType.SP
                 and x.sync_info is not None
                 and all('barrier' not in (w.ant_name or '')
                         for w in x.sync_info.on_wait)]
    _long_pos = None
    for _i in _sp_waits:
        if any((w.wait_value or 0) >= 48 for w in _exit2[_i].sync_info.on_wait):
            _long_pos = _i
    if _long_pos is not None:
        for _i in _sp_waits:
            if _i > _long_pos:
                _exit_blk2.instructions.remove(_exit2[_i])
    return nc


def _host_prep(final_pred, step_preds, uncertainty, area_targets,
               recipe_embeddings):
    final_pred = np.asarray(final_pred, dtype=np.float32)
    step_preds = np.asarray(step_preds, dtype=np.float32)
    uncertainty = np.asarray(uncertainty, dtype=np.float32)
    area_targets = np.asarray(area_targets, dtype=np.float32)
    e = np.asarray(recipe_embeddings, dtype=np.float32)

    e8 = (e * (1.0 / ESCALE)).astype(FP8NP)
    # bit-double safety: no byte may sit at |bits|==0x3F (doubles to the
    # fp8 NaN pattern) or >=0x40 (|e/4|>=2, doubles into the sign bit)
    eb = e8.view(np.uint8)
    bad = (eb & 0x7F) >= 0x3F
    if bad.any():
        eb[bad] = (eb[bad] & 0x80) | 0x3E
    e8 = e8.reshape(B, NCH, 128, D).transpose(0, 2, 1, 3)
    e8 = np.ascontiguousarray(e8).reshape(B, 128, NCH * D)

    maps = []
    for i in range(NCORES):
        s = slice(i * BS, (i + 1) * BS)
        sp = step_preds[s]
        at = area_targets[s]
        # flat overlap layout [128, 2, 257]: col j = flat[p*256 + j - 1],
        # with col 0 at batch starts duplicated (= flat[p*256])
        spat_f = np.empty((128, 2, FP_ + 1), np.float32)
        for t_i, x in enumerate((sp, at)):
            flat = x.reshape(-1)
            spat_f[:, t_i, 1:] = flat.reshape(128, FP_)
            spat_f[1:, t_i, 0] = flat[FP_ - 1:FLAT - 1:FP_]
            spat_f[0:128:4, t_i, 0] = flat[0::L]
        # [k, 2, (b,c)] bf16 layout for weight prep
        spat_t = np.empty((128, 2, FP_), np.float32)
        for t_i, x in enumerate((sp, at)):
            # x[b, c*128+k] -> spat_t[k, t, 8b+c]
            spat_t[:, t_i, :] = x.reshape(BS, NCH, 128).transpose(
                2, 0, 1).reshape(128, FP_)
        smalls = np.stack([
            final_pred[s, 0], uncertainty[s, 0], at[:, L - 1],
            sp[:, 0], at[:, 0], np.zeros(BS, np.float32)], axis=1)
        maps.append({
            "emb": np.ascontiguousarray(e8[s]),
            "spat_t": np.ascontiguousarray(
                spat_t.reshape(128, -1).astype(FP8NP)),
            "spat_f": np.ascontiguousarray(spat_f.reshape(128, -1)),
            "smalls": np.ascontiguousarray(smalls),
        })
    return maps


def _combine(results):
    p = np.stack([
        np.asarray(r["partials"], dtype=np.float64).sum(axis=0)
        for r in results])
    tot = p.sum(axis=0)
    s_fd2, s_step, s_rel, s_cA, s_cB, s_unc = tot[0:6]
    s_s2 = tot[6:10].sum()
    s_qw = tot[10]
    s_cj = tot[11]
    final_loss = s_fd2 / B
    step_loss = s_step / (B * L)
    rel_loss = s_rel / (B * (L - 1))
    crit_loss = (s_cA - s_cj + s_cB) / (B * (L - 1))
    seq_dep = step_loss + (s_s2 * S2SCALE - s_qw * QSCALE) / 2.0 / (B * L)
    unc_loss = 0.5 * s_unc / B
    total = (final_loss + rel_loss + step_loss
             + 0.3 * crit_loss + 0.2 * seq_dep + 0.3 * unc_loss)
    return np.float32(total)


def _run(in_maps, trace=False, **kw):
    if "nc" not in _CACHE:
        _CACHE["nc"] = _build_nc()
    return run_bass_kernel_spmd(
        _CACHE["nc"], in_maps, core_ids=list(range(NCORES)), trace=trace, **kw)


def kernel(final_pred, step_preds, uncertainty, area_targets,
           recipe_embeddings, recipes=None, **_ignored):
    maps = _host_prep(final_pred, step_preds, uncertainty, area_targets,
                      recipe_embeddings)
    results = _run(maps).results
    return _combine(results)


if __name__ == "__main__":
    import os
    import time
    import reference
    inputs = {k: np.asarray(v) for k, v in reference.setup_inputs().items()}
    t0 = time.time()
    actual = kernel(**inputs)
    print(f"kernel4: {actual}  ({time.time() - t0:.1f}s)")
    cache = "/root/problem/_expected_cache.npz"
    if os.path.exists(cache):
        expected = np.load(cache)["expected"]
    else:
        expected = np.asarray(reference.reference(**inputs))
    rel = abs(float(actual) - float(expected)) / abs(float(expected))
    print(f"expected: {expected}  rel: {rel:.3e}")
    from concourse.timeline_sim import TimelineSim
    t_ns = TimelineSim(_CACHE["nc"], trace=False).simulate()
    print(f"HW exec time: {t_ns:.0f} ns")


# revision 9
# speedup vs baseline: 1.0097x; 1.0097x over previous
"""Bass/Trainium2 kernel v4 for nn_EnhancedCircuitLoss — DMA-bound stream.

Math: the O(B*L^2*D) dep term collapses to per-batch S = sum_l sq_l e_l and
Q = sum_l sq_l^2 |e_l|^2 via sum_b sq.dep = (|S_b|^2 - Q_b)/2.

v4 (30.5us/core, from 46.7us v3). The 8MB/core fp8 embedding stream is the
roofline (23.3us at 360 B/ns); everything else hides under it or is fixed
framework edge cost (entry chain ~1.3us, exit handshakes ~4.9us, both
trimmed by pre/post-compile BIR surgery in _build_nc):
- emb stored host-side as fp8(e/4) so every byte has exponent < 8; batches
  arrive in 4-batch DMAs (tapered 2/1/quartered at the tail) — 13 input
  DMAs total instead of 35+, so HWDGE/SEQ overheads and the per-DMA
  completion sems stay off the stream.
- Q squares: one DVE tensor_scalar logical_shift_left(1) on a uint16 view
  per batch (422ns at the 4x DVE rate). Doubling fp8 bit patterns squares
  the values up to a 2^7 scale (folded into host combine) with <=9%
  elementwise error (mantissa interpolation + cross-byte carry), far
  inside this term's error budget; e/4 keeps 2E < 16 so no sign/NaN
  overflow. The e2' stream feeds DoubleRow wred matmuls (wt sq^2/16)
  accumulated across all batches in one PSUM bank (row 0 read once).
- S: DoubleRow matmuls with guard-spaced scattered sq weights (stride 68
  per batch, 16 per chunk-pair, double-buffered 784-col regions) so each
  8-batch group's S vectors land on DISTINCT PSUM partitions (0..7) of
  one accumulated bank; |S|^2 is a single [8,256] ACT Square+accum per
  group, overlapped with the stream (last group parallel with the DVE
  qwr drain).
- small terms in the flat [128, 2, 257] overlap layout (boundary dups);
  weight prep from an fp8 [128, 2, 256] (k, (b,c)) layout — no PE
  transposes, no big memsets (zeros live once in the 2x784 guard tile).
"""

import numpy as np
import ml_dtypes

import concourse.bacc as bacc
import concourse.bass as bass
import concourse.mybir as mybir
import concourse.tile as tile
from concourse.bass_utils import run_bass_kernel_spmd

F32 = mybir.dt.float32
BF16 = mybir.dt.bfloat16
FP8 = mybir.dt.float8e4
U16 = mybir.dt.uint16
FP8NP = ml_dtypes.float8_e4m3
ALU = mybir.AluOpType
AF = mybir.ActivationFunctionType

B, L, D = 256, 1024, 256
NCORES = 8
BS = B // NCORES          # 32 batches per core
NCH = L // 128            # 8 l-chunks of 128
NG = NCH // 2             # 4 double-chunks for DoubleRow
EPS = 1e-6
FLAT = BS * L
FP_ = FLAT // 128         # 256 per partition

# emb stored as e/4 (keeps fp8 exponent < 8 so bit-doubling can't spill
# into the sign bit); squares come back as (e/4)^2 * 2^7 = e^2 / 2.
ESCALE = 4.0
S2SCALE = ESCALE * ESCALE          # |S|^2 = S2SCALE * raw
QSCALE = 2.0                       # Q = 16(sq2) * ESCALE^2 / 2^7 * raw

# DMA batch grouping: big groups for BW, tapered tail for latency.
GROUPS = [4, 4, 4, 4, 4, 4, 4, 2, 1, "q"]
assert sum(g if g != "q" else 1 for g in GROUPS) == BS

_CACHE = {}


def _build_nc():
    nc = bacc.Bacc("TRN2", target_bir_lowering=False, debug=False)

    emb = nc.dram_tensor("emb", [BS, 128, NCH * D], FP8, kind="ExternalInput")
    spat_t = nc.dram_tensor("spat_t", [128, 2 * FP_], FP8, kind="ExternalInput")
    spat_f = nc.dram_tensor("spat_f", [128, 2 * (FP_ + 1)], F32,
                            kind="ExternalInput")
    smalls = nc.dram_tensor("smalls", [BS, 6], F32, kind="ExternalInput")
    partials = nc.dram_tensor("partials", [128, 16], F32, kind="ExternalOutput")

    with tile.TileContext(nc) as tc:
        with (
            tc.tile_pool(name="persist", bufs=1) as pp,
            tc.tile_pool(name="ebuf", bufs=4) as ep,
            tc.tile_pool(name="e2buf", bufs=4) as e2p,
            tc.tile_pool(name="psum", bufs=1, space=bass.MemorySpace.PSUM) as ps,
        ):
            # re-init the one const AP the stripped preamble memsets fed
            # (activation bias 0.0); ordered before all ACT Squares via the
            # DVE->ACT sqd_t sem chain
            nc.vector.memset(nc.const_aps.aps[(F32, 0.0)], 0.0)

            # ---------- input DMAs (order = DMA device order) ----------
            spatt_b = pp.tile([128, 2, FP_], FP8, tag="spatt_b")
            spat_b = pp.tile([128, 2, FP_ + 1], F32, tag="spat_b")
            smalls_b = pp.tile([BS, 6], F32, tag="smalls_b")

            ebufs = []
            off = 0
            group_of_batch = {}
            for gi, gsz in enumerate(GROUPS):
                if gsz == "q":
                    # final batch: 4 quarter-DMAs (2 chunks each) so the
                    # tail shl/wred pipeline with the transfer
                    t = ep.tile([128, 1, NCH, D], FP8, tag="ebufq")
                    for h in range(4):
                        nc.sync.dma_start(
                            t[:, 0, 2 * h:2 * h + 2, :],
                            emb.ap()[off].rearrange(
                                "k (c d) -> k c d", c=NCH)[:, 2 * h:2 * h + 2, :])
                    group_of_batch[off] = (t, 0)
                    off += 1
                    continue
                t = ep.tile([128, gsz, NCH, D], FP8, tag=f"ebuf{gsz}")
                nc.sync.dma_start(
                    t[:], emb.ap()[off:off + gsz].rearrange(
                        "b k (c d) -> k b c d", c=NCH))
                for j in range(gsz):
                    group_of_batch[off + j] = (t, j)
                off += gsz
                if gi == 0:
                    nc.sync.dma_start(
                        spatt_b[:],
                        spat_t.ap().rearrange("p (s j) -> p s j", s=2))
                elif gi == 1:
                    nc.sync.dma_start(
                        spat_b[:],
                        spat_f.ap().rearrange("p (s j) -> p s j", s=2))
                    nc.sync.dma_start(smalls_b[:], smalls.ap())

            # ---------- weight prep ----------
            # sq8w: double-buffered guard-spaced scatter regions (784 each).
            # DoubleRow out row p reads lhsT cols [254-2p, 255-2p]; batch b8
            # of the active group lands its S row at partition b8: value
            # cols V = 254 + 66*b8 + 16*g + j, window x = 68*b8 + 16*g.
            sq8w = pp.tile([128, 2, 784], FP8, tag="sq8w")
            nc.gpsimd.memset(sq8w[:].rearrange("p a b -> p (a b)"), 0.0)
            # wred weights: sliding window x=8b+2g over [254 zeros | sq2T]
            sq2pad = pp.tile([128, 510], FP8, tag="sq2pad")
            nc.gpsimd.memset(sq2pad[:, 0:254], 0.0)

            sqd_t = pp.tile([128, FP_], BF16, tag="sqd_t")
            nc.vector.tensor_sub(sqd_t[:], spatt_b[:, 0, :], spatt_b[:, 1, :])
            sq8T = pp.tile([128, FP_], FP8, tag="sq8T")
            nc.scalar.activation(sq8T[:], sqd_t[:], AF.Square)
            nc.scalar.activation(sq2pad[:, 254:510], sq8T[:], AF.Square,
                                 scale=0.25)

            def scatter(q):
                # values at cols 240 + 66*b8 + 16*g + j of region q%2,
                # from sq8T cols 64q + 8*b8 + 2*g + j
                for j in range(2):
                    src = sq8T[:, 64 * q + j:64 * q + j + 63:2].rearrange(
                        "p (b g) -> p b g", b=8)
                    dst = sq8w[:, q % 2, 254 + j:254 + j + 66 * 8:1].rearrange(
                        "p (b r) -> p b r", b=8)[:, :, 0:49:16]
                    nc.vector.tensor_copy(dst, src)

            scatter(0)
            scatter(1)

            # ---------- persistent tiles for small terms ----------
            spB = spat_b[:, 0, :]
            atB = spat_b[:, 1, :]
            sqd_f = pp.tile([128, FP_], F32, tag="sqd_f")
            d_at = pp.tile([128, FP_], F32, tag="d_at")
            spe = pp.tile([128, FP_ + 1], F32, tag="spe")
            ate = pp.tile([128, FP_ + 1], F32, tag="ate")
            rsp = pp.tile([128, FP_], F32, tag="rsp")
            rat = pp.tile([128, FP_], F32, tag="rat")
            t1 = pp.tile([128, FP_], F32, tag="t1")
            t2 = pp.tile([128, FP_], F32, tag="t2")
            rdiff = pp.tile([128, FP_], F32, tag="rdiff")
            s2scr = pp.tile([128, 256], BF16, tag="s2scr")
            prodB = pp.tile([128, FP_], F32, tag="prodB")
            sq_scr = pp.tile([128, FP_], BF16, tag="sq_scr")

            finals = pp.tile([128, 16], F32, tag="finals")
            nc.gpsimd.memset(finals[:], 0.0)

            # DVE ops to interleave into stream gaps (cheap-first order
            # matters only for dependencies)
            dve_ops = [
                lambda: nc.vector.tensor_sub(sqd_f[:], spB[:, 1:], atB[:, 1:]),
                lambda: nc.vector.tensor_scalar(spe[:], spB[:], EPS, None,
                                                ALU.add),
                lambda: nc.vector.tensor_scalar(ate[:], atB[:], EPS, None,
                                                ALU.add),
                lambda: nc.vector.reciprocal(rsp[:], spe[:, 0:FP_]),
                lambda: nc.vector.reciprocal(rat[:], ate[:, 0:FP_]),
                lambda: nc.vector.tensor_mul(t1[:], spe[:, 1:], rsp[:]),
                lambda: nc.vector.tensor_mul(t2[:], ate[:, 1:], rat[:]),
                lambda: nc.vector.tensor_sub(rdiff[:], t1[:], t2[:]),
                lambda: nc.vector.tensor_sub(d_at[:], atB[:, 1:], atB[:, 0:FP_]),
                lambda: nc.vector.tensor_mul(prodB[:], sqd_f[:], d_at[:]),
                lambda: nc.vector.tensor_reduce(
                    finals[:, 3:4], sqd_f[:], mybir.AxisListType.X,
                    ALU.add, apply_absolute_value=True),
                lambda: nc.vector.tensor_reduce(
                    finals[:, 4:5], prodB[:], mybir.AxisListType.X,
                    ALU.add, apply_absolute_value=True),
            ]
            act_ops = {
                10: lambda: nc.scalar.activation(
                    sq_scr[:], sqd_f[:], AF.Square, accum_out=finals[:, 1:2]),
                20: lambda: nc.scalar.activation(
                    sq_scr[:], rdiff[:], AF.Square, accum_out=finals[:, 2:3]),
            }

            # Pool-engine small chain (off critical path)
            fd = pp.tile([BS, 1], F32, tag="fd")
            lg = pp.tile([BS, 1], F32, tag="lg")
            invu = pp.tile([BS, 1], F32, tag="invu")
            tJ = pp.tile([BS, 1], F32, tag="tJ")
            nc.scalar.activation(lg[:], smalls_b[:, 1:2], AF.Ln)
            nc.vector.reciprocal(invu[:], smalls_b[:, 1:2])
            nc.gpsimd.tensor_sub(fd[:], smalls_b[:, 0:1], smalls_b[:, 2:3])
            nc.gpsimd.tensor_mul(finals[0:BS, 0:1], fd[:], fd[:])
            nc.gpsimd.tensor_mul(finals[0:BS, 5:6], finals[0:BS, 0:1], invu[:])
            nc.gpsimd.tensor_add(finals[0:BS, 5:6], finals[0:BS, 5:6], lg[:])
            nc.gpsimd.tensor_sub(tJ[:], smalls_b[:, 3:4], smalls_b[:, 4:5])
            nc.vector.tensor_reduce(finals[0:BS, 11:12], tJ[:],
                                    mybir.AxisListType.X, ALU.add,
                                    apply_absolute_value=True)

            # ---------- stream ----------
            # full-bank (2KB) PSUM tiles so concurrent accumulation groups
            # never share a bank
            sbanks = []
            for q in range(4):
                sbank_t = ps.tile([128, 512], F32, tag=f"sbank{q}")
                sbanks.append(sbank_t[:, 0:256])
            qwr_t = ps.tile([128, 512], F32, tag="qwr")
            qwr = qwr_t[:, 0:256]

            for b in range(BS):
                t, j = group_of_batch[b]
                q = b // 8
                b8 = b % 8

                def s_mm(g):
                    x = 784 * (q % 2) + 68 * b8 + 16 * g
                    nc.tensor.matmul(
                        sbanks[q][:],
                        sq8w[:].rearrange("p a b -> p (a b)")[:, x:x + 256],
                        t[:, j, 2 * g:2 * g + 2, :],
                        start=(b8 == 0 and g == 0),
                        stop=(b8 == 7 and g == NG - 1),
                        perf_mode=mybir.MatmulPerfMode.DoubleRowSwInterleave,
                        skip_group_check=True)

                def wred_mm(g, e2):
                    x = 8 * b + 2 * g
                    nc.tensor.matmul(
                        qwr[:], sq2pad[:, x:x + 256],
                        e2[:, 2 * g:2 * g + 2, :],
                        start=(b == 0 and g == 0),
                        stop=(b == BS - 1 and g == NG - 1),
                        perf_mode=mybir.MatmulPerfMode.DoubleRowSwInterleave,
                        skip_group_check=True)

                e2 = e2p.tile([128, NCH, D], FP8, tag="e2")
                if b == BS - 1:
                    # quartered tail batch: S-mm, shl, wred per chunk-pair
                    for h in range(NG):
                        s_mm(h)
                        sl = slice(2 * h, 2 * h + 2)
                        nc.vector.tensor_scalar(
                            e2[:, sl].rearrange("p c d -> p (c d)").bitcast(U16),
                            t[:, j, sl].rearrange("p c d -> p (c d)").bitcast(U16),
                            1, None, ALU.logical_shift_left)
                        wred_mm(h, e2)
                else:
                    for g in range(NG):
                        s_mm(g)
                    nc.vector.tensor_scalar(
                        e2[:].rearrange("p c d -> p (c d)").bitcast(U16),
                        t[:, j].rearrange("p c d -> p (c d)").bitcast(U16),
                        1, None, ALU.logical_shift_left)
                    for g in range(NG):
                        wred_mm(g, e2)

                if b8 == 7:
                    # |S|^2 for the finished group: rows 0..7
                    nc.scalar.activation(
                        s2scr[0:8, :], sbanks[q][0:8, :], AF.Square,
                        accum_out=finals[0:8, 6 + q:7 + q])
                    if q + 2 < 4:
                        scatter(q + 2)

                if b >= 4 and dve_ops:
                    dve_ops.pop(0)()
                if b in act_ops:
                    act_ops.pop(b)()

            for f in dve_ops:
                f()
            for f in act_ops.values():
                f()

            # Q drain: row 0 of qwr, on DVE (parallel with ACT q3 square)
            nc.vector.tensor_reduce(finals[0:1, 10:11], qwr[0:1, :],
                                    mybir.AxisListType.X, ALU.add)
            nc.sync.dma_start(partials.ap(), finals[:])

    # strip all 4 const-AP preamble memsets (they serialize on Pool ahead of
    # the entry barrier); the one const this kernel reads (activation bias
    # 0.0) is re-initialized in the body on DVE, ordered before every ACT
    # Square by the DVE->ACT sqd_t sem chain
    blk0 = nc.m.functions[0].blocks[0]
    _memsets = [i for i in blk0.instructions if isinstance(i, mybir.InstMemset)][:4]
    for _ins in _memsets:
        blk0.instructions.remove(_ins)
    # hoist the first emb DMA (no data deps) ahead of the entry barrier so
    # its transfer overlaps the barrier round-trip
    _first_dma = None
    _blk_of = None
    for _blk in nc.m.functions[0].blocks:
        for _i in _blk.instructions:
            if isinstance(_i, mybir.InstDMACopy):
                _first_dma = _i
                _blk_of = _blk
                break
        if _first_dma is not None:
            break
    _blk_of.instructions.remove(_first_dma)
    blk0.instructions.insert(1, _first_dma)
    # drop the SECOND exit barrier round (drains + barriers after the Pool
    # sem-clear ISA); round 1 already orders the sem-clear after all engines'
    # completion waits, and exit-barrier sems are untouched either way
    _exit_blk = nc.m.functions[0].blocks[-1]
    _exit = list(_exit_blk.instructions)
    _isa_idx = max(i for i, x in enumerate(_exit)
                   if isinstance(x, mybir.InstISA))
    for _ins in _exit[_isa_idx + 1:]:
        _exit_blk.instructions.remove(_ins)
    nc.compile()
    # the DMA-queue completion waits are generated BY compile(); drop the SP
    # exit waits queued after the long output-DMA wait (input queues that
    # completed mid-stream + engine tick sems already guarded by the exit
    # drains) so the barrier+sem-clear start as soon as the output lands
    _exit_blk2 = nc.m.functions[0].blocks[-1]
    _exit2 = list(_exit_blk2.instructions)
    _sp_waits = [i for i, x in enumerate(_exit2)
                 if isinstance(x, mybir.InstEventSemaphore)
                 and x.engine == mybir.EngineType.SP
                 and x.sync_info is not None
                 and all('barrier' not in (w.ant_name or '')
                         for w in x.sync_info.on_wait)]
    _long_pos = None
    for _i in _sp_waits:
        if any((w.wait_value or 0) >= 48 for w in _exit2[_i].sync_info.on_wait):
            _long_pos = _i
    if _long_pos is not None:
        for _i in _sp_waits:
            if _i > _long_pos:
                _exit_blk2.instructions.remove(_exit2[_i])
    # retarget the sem-clear ISA to SP: SP's in-order queue then guarantees
    # it runs after the output-completion wait, making the exit barrier
    # round (and its drains) unnecessary
    _exit3 = list(_exit_blk2.instructions)
    _isa = [x for x in _exit3 if isinstance(x, mybir.InstISA)]
    if len(_isa) == 1:
        _isa[0].engine = mybir.EngineType.SP
        for x in _exit3:
            if isinstance(x, mybir.InstDrain) or (
                    isinstance(x, mybir.InstEventSemaphore)
                    and x.sync_info is not None
                    and any('barrier' in (w.ant_name or '')
                            for w in list(x.sync_info.on_wait)
                            + list(x.sync_info.on_update))):
                _exit_blk2.instructions.remove(x)
    return nc


def _host_prep(final_pred, step_preds, uncertainty, area_targets,
               recipe_embeddings):
    final_pred = np.asarray(final_pred, dtype=np.float32)
    step_preds = np.asarray(step_preds, dtype=np.float32)
    uncertainty = np.asarray(uncertainty, dtype=np.float32)
    area_targets = np.asarray(area_targets, dtype=np.float32)
    e = np.asarray(recipe_embeddings, dtype=np.float32)

    e8 = (e * (1.0 / ESCALE)).astype(FP8NP)
    # bit-double safety: no byte may sit at |bits|==0x3F (doubles to the
    # fp8 NaN pattern) or >=0x40 (|e/4|>=2, doubles into the sign bit)
    eb = e8.view(np.uint8)
    bad = (eb & 0x7F) >= 0x3F
    if bad.any():
        eb[bad] = (eb[bad] & 0x80) | 0x3E
    e8 = e8.reshape(B, NCH, 128, D).transpose(0, 2, 1, 3)
    e8 = np.ascontiguousarray(e8).reshape(B, 128, NCH * D)

    maps = []
    for i in range(NCORES):
        s = slice(i * BS, (i + 1) * BS)
        sp = step_preds[s]
        at = area_targets[s]
        # flat overlap layout [128, 2, 257]: col j = flat[p*256 + j - 1],
        # with col 0 at batch starts duplicated (= flat[p*256])
        spat_f = np.empty((128, 2, FP_ + 1), np.float32)
        for t_i, x in enumerate((sp, at)):
            flat = x.reshape(-1)
            spat_f[:, t_i, 1:] = flat.reshape(128, FP_)
            spat_f[1:, t_i, 0] = flat[FP_ - 1:FLAT - 1:FP_]
            spat_f[0:128:4, t_i, 0] = flat[0::L]
        # [k, 2, (b,c)] bf16 layout for weight prep
        spat_t = np.empty((128, 2, FP_), np.float32)
        for t_i, x in enumerate((sp, at)):
            # x[b, c*128+k] -> spat_t[k, t, 8b+c]
            spat_t[:, t_i, :] = x.reshape(BS, NCH, 128).transpose(
                2, 0, 1).reshape(128, FP_)
        smalls = np.stack([
            final_pred[s, 0], uncertainty[s, 0], at[:, L - 1],
            sp[:, 0], at[:, 0], np.zeros(BS, np.float32)], axis=1)
        maps.append({
            "emb": np.ascontiguousarray(e8[s]),
            "spat_t": np.ascontiguousarray(
                spat_t.reshape(128, -1).astype(FP8NP)),
            "spat_f": np.ascontiguousarray(spat_f.reshape(128, -1)),
            "smalls": np.ascontiguousarray(smalls),
        })
    return maps


def _combine(results):
    p = np.stack([
        np.asarray(r["partials"], dtype=np.float64).sum(axis=0)
        for r in results])
    tot = p.sum(axis=0)
    s_fd2, s_step, s_rel, s_cA, s_cB, s_unc = tot[0:6]
    s_s2 = tot[6:10].sum()
    s_qw = tot[10]
    s_cj = tot[11]
    final_loss = s_fd2 / B
    step_loss = s_step / (B * L)
    rel_loss = s_rel / (B * (L - 1))
    crit_loss = (s_cA - s_cj + s_cB) / (B * (L - 1))
    seq_dep = step_loss + (s_s2 * S2SCALE - s_qw * QSCALE) / 2.0 / (B * L)
    unc_loss = 0.5 * s_unc / B
    total = (final_loss + rel_loss + step_loss
             + 0.3 * crit_loss + 0.2 * seq_dep + 0.3 * unc_loss)
    return np.float32(total)


def _run(in_maps, trace=False, **kw):
    if "nc" not in _CACHE:
        _CACHE["nc"] = _build_nc()
    return run_bass_kernel_spmd(
        _CACHE["nc"], in_maps, core_ids=list(range(NCORES)), trace=trace, **kw)


def kernel(final_pred, step_preds, uncertainty, area_targets,
           recipe_embeddings, recipes=None, **_ignored):
    maps = _host_prep(final_pred, step_preds, uncertainty, area_targets,
                      recipe_embeddings)
    results = _run(maps).results
    return _combine(results)


if __name__ == "__main__":
    import os
    import time
    import reference
    inputs = {k: np.asarray(v) for k, v in reference.setup_inputs().items()}
    t0 = time.time()
    actual = kernel(**inputs)
    print(f"kernel4: {actual}  ({time.time() - t0:.1f}s)")
    cache = "/root/problem/_expected_cache.npz"
    if os.path.exists(cache):
        expected = np.load(cache)["expected"]
    else:
        expected = np.asarray(reference.reference(**inputs))
    rel = abs(float(actual) - float(expected)) / abs(float(expected))
    print(f"expected: {expected}  rel: {rel:.3e}")
    from concourse.timeline_sim import TimelineSim
    t_ns = TimelineSim(_CACHE["nc"], trace=False).simulate()
    print(f"HW exec time: {t_ns:.0f} ns")
